# revision 48
# baseline (speedup 1.0000x reference)
"""4-layer GCN (GroupGCN) on 8 TRN2 NeuronCores.

Strategy:
  - Nodes relabeled by degree rank and striped across the 8 cores (core = rank % 8)
    so every core owns 12500 nodes with an even mix of degrees; edges are assigned
    to the core that owns their dst node.
  - Per layer: each core computes the scaled feature table rows for its own nodes
    (h~ = dinv * (z @ W)) with PE matmuls; the table is distributed to all cores
    with CHUNKED AllGathers (4 group-range chunks, chunk-major DRAM rows) that
    are issued progressively from inside the previous layer's aggregation loop,
    so collectives and matmuls hide under the gather stream.
  - Aggregation: one indirect-DMA gather per supergroup (groups packed to a
    budget of 224 slots, no trailing padding) + a DVE fold-tree segmented sum
    (bf16 for 3 levels, f32 scratch after).
  - dinv (D^-1/2) factorizes out of the per-edge norm: table rows are pre-scaled
    by dinv[src], the aggregated sum is post-scaled by dinv[dst] (fused with the
    ReLU on the Scalar engine when biases are all-zero).
  - bf16 tables/matmuls, f32 accumulation, f32 log_softmax output.
"""

import numpy as np
import ml_dtypes

N_NODES = 100000
N_EDGES = 3200000
IN_DIM = 256
DIMS = [64, 32, 16, 4]
C = 8           # cores
P = 128         # partitions
G = 98          # node groups of 128 per core
PC = G * P      # padded nodes per core (12544)
NPC = N_NODES // C  # real nodes per core (12500)
MSG_SLOT_BUDGET = 224   # max slots (w*K) per supergroup gather
CHUNKS = [0, 24, 48, 70, 88, 98]   # table chunk boundaries (group ranges)
NCH = len(CHUNKS) - 1

_COMPILED = {}
LAST_RESULT = None


# ----------------------------------------------------------------------------
# Host-side graph preprocessing
# ----------------------------------------------------------------------------
def _row_of_pos():
    """DRAM table row for each (pos in [0, PC)): chunk-major layout.

    Chunk c spans groups [g0, g1); global rows for the chunk are
    [C*P*g0, C*P*g1) with row = C*P*g0 + core*P*w + p*w + (g-g0)."""
    pos = np.arange(PC)
    g = pos // P
    p = pos % P
    row = np.empty(PC, dtype=np.int64)
    for ci in range(len(CHUNKS) - 1):
        g0, g1 = CHUNKS[ci], CHUNKS[ci + 1]
        w = g1 - g0
        m = (g >= g0) & (g < g1)
        # core offset added later (core stride inside chunk is P*w)
        row[m] = C * P * g0 + p[m] * w + (g[m] - g0)
    return row  # per-core row; add core*P*w_chunk via chunk lookup


def _preprocess(edge_index):
    src = edge_index[0].astype(np.int64)
    dst = edge_index[1].astype(np.int64)
    loop = np.arange(N_NODES, dtype=np.int64)
    src = np.concatenate([src, loop])
    dst = np.concatenate([dst, loop])

    deg = np.bincount(dst, minlength=N_NODES).astype(np.float64)
    dinv = np.where(deg > 0, 1.0 / np.sqrt(deg), 0.0).astype(np.float32)

    order = np.argsort(deg, kind="stable")          # ascending degree
    core_of_rank = np.arange(N_NODES) % C
    pos_of_rank = np.arange(N_NODES) // C
    new_id = np.empty(N_NODES, dtype=np.int64)      # (c, pos) packed
    new_id[order] = core_of_rank * PC + pos_of_rank

    # global table row for every (core, pos)
    pos_all = np.arange(C * PC) % PC
    core_all = np.arange(C * PC) // PC
    g_all = pos_all // P
    p_all = pos_all % P
    row_all = np.empty(C * PC, dtype=np.int64)
    for ci in range(len(CHUNKS) - 1):
        g0, g1 = CHUNKS[ci], CHUNKS[ci + 1]
        w = g1 - g0
        m = (g_all >= g0) & (g_all < g1)
        row_all[m] = (C * P * g0 + core_all[m] * P * w + p_all[m] * w
                      + (g_all[m] - g0))

    old_of_row = np.full(C * PC, -1, dtype=np.int64)
    old_of_row[row_all[new_id]] = np.arange(N_NODES)

    src_n = new_id[src]
    dst_n = new_id[dst]

    sort_idx = np.lexsort((src_n, dst_n))
    src_n = src_n[sort_idx]
    dst_n = dst_n[sort_idx]

    deg_n = np.bincount(dst_n, minlength=C * PC)
    deg_pg = deg_n.reshape(C, G, P)
    K_g = np.maximum(deg_pg.max(axis=(0, 2)), 1).astype(np.int64)  # [G]

    # pack groups into supergroups of at most S0 = w*K slots (compact: the
    # gather width is exactly w*K, no budget padding)
    S0 = MSG_SLOT_BUDGET
    sgs = []  # list of (g_start, width, K)
    g0 = 0
    while g0 < G:
        K = int(K_g[g0])
        w = 1
        while (g0 + w < G and
               max(K, int(K_g[g0 + w])) * (w + 1) <= S0 and
               w < 14):
            K = max(K, int(K_g[g0 + w]))
            w += 1
        K = max(K, int(K_g[g0:g0 + w].max()))
        sgs.append((g0, w, K))
        g0 += w

    col0 = np.zeros(len(sgs) + 1, dtype=np.int64)   # idx col offset per sg
    for si, (gs, w, K) in enumerate(sgs):
        col0[si + 1] = col0[si] + w * K
    SK = int(col0[-1])
    col_of_group = np.zeros(G, dtype=np.int64)
    for si, (gs, w, K) in enumerate(sgs):
        for i in range(w):
            col_of_group[gs + i] = col0[si] + i * K

    # zero row: first dummy node of core 0 (table rows of dummies are zeroed
    # by their dinv=0 pre-scale)
    ZERO_ROW = int(row_all[NPC])

    IDX = np.full((C, P, SK), ZERO_ROW, dtype=np.int32)
    node_starts = np.zeros(C * PC + 1, dtype=np.int64)
    np.cumsum(deg_n, out=node_starts[1:])
    slot_in_node = np.arange(len(dst_n)) - node_starts[dst_n]
    core_e = dst_n // PC
    pos_e = dst_n % PC
    p_e = pos_e % P
    g_e = pos_e // P
    col_e = col_of_group[g_e] + slot_in_node
    IDX[core_e, p_e, col_e] = row_all[src_n].astype(np.int32)

    dinv_n = np.zeros(C * PC, dtype=np.float32)
    dinv_n[new_id] = dinv
    dinv_arr = dinv_n.reshape(C, G, P).transpose(0, 2, 1).copy()  # [C, P, G]

    return dict(
        dinv_arr=dinv_arr, IDX=IDX, sgs=sgs, SK=SK,
        old_of_row=old_of_row, new_id=new_id, dinv=dinv,
    )


# ----------------------------------------------------------------------------
# Device kernel builder
# ----------------------------------------------------------------------------
def _build_program(sgs, SK, b_zero):
    import concourse.bass as bass
    import concourse.mybir as mybir
    import concourse.tile as tile
    import concourse.bacc as bacc
    from concourse.masks import make_identity
    from concourse.bass import _add_dep_helper

    fp32 = mybir.dt.float32
    bf16 = mybir.dt.bfloat16
    i32 = mybir.dt.int32
    AF = mybir.ActivationFunctionType
    ALU = mybir.AluOpType

    nc = bacc.Bacc("TRN2", target_bir_lowering=False, debug=False,
                   enable_asserts=False, num_devices=C)

    dims = [IN_DIM] + DIMS
    GB = {1: 2, 2: 4, 3: 4}   # matmul group-batch per layer (block-diag W)
    xT = nc.dram_tensor("xT", [IN_DIM, PC], bf16, kind="ExternalInput").ap()
    idx_d = nc.dram_tensor("idx", [P, SK], i32, kind="ExternalInput").ap()
    dinv_d = nc.dram_tensor("dinv", [P, G], fp32, kind="ExternalInput").ap()
    dinv2_d = nc.dram_tensor("dinv2", [P, G], fp32, kind="ExternalInput").ap()
    wblk_d = {}
    for l in (1, 2, 3):
        wblk_d[l] = nc.dram_tensor(
            f"wblk{l+1}", [GB[l] * dims[l], GB[l] * dims[l + 1]], bf16,
            kind="ExternalInput").ap()
    w_d = []
    b_d = []
    for l in range(4):
        w_d.append(nc.dram_tensor(f"w{l+1}", [dims[l], dims[l + 1]], bf16,
                                  kind="ExternalInput").ap())
        b_d.append(nc.dram_tensor(f"b{l+1}", [P, dims[l + 1]], fp32,
                                  kind="ExternalInput").ap())
    out_d = nc.dram_tensor("out", [PC, 4], fp32, kind="ExternalOutput").ap()

    with tile.TileContext(nc) as tc:
        with (
            tc.tile_pool(name="const", bufs=1) as cpool,
            tc.tile_pool(name="work", bufs=1) as wpool,
            tc.tile_pool(name="msg", bufs=3) as mpool,
            tc.tile_pool(name="mm", bufs=3) as mmpool,
            tc.tile_pool(name="psum", bufs=4, space="PSUM") as pspool,
            tc.tile_pool(name="pst", bufs=4, space="PSUM") as pstpool,
            tc.tile_pool(name="dram", bufs=1, space="DRAM") as dpool,
        ):
            # ---- constants ----
            dinv_sb = cpool.tile([P, G], fp32)
            nc.sync.dma_start(dinv_sb[:], dinv_d[:])
            dinv2_sb = cpool.tile([P, G], fp32)
            nc.sync.dma_start(dinv2_sb[:], dinv2_d[:])
            wblk_sb = {}
            for l in (1, 2, 3):
                wblk_sb[l] = cpool.tile(
                    [GB[l] * dims[l], GB[l] * dims[l + 1]], bf16,
                    tag=f"wblk{l}", name=f"wblk{l}")
                nc.sync.dma_start(wblk_sb[l][:], wblk_d[l][:])
            idx_sb = cpool.tile([P, SK], i32, tag="idxsb")
            nc.sync.dma_start(idx_sb[:], idx_d[:])
            ident = cpool.tile([P, P], bf16)
            make_identity(nc, ident[:])
            w_sb = []
            b_sb = []
            for l in range(4):
                din, dout = dims[l], dims[l + 1]
                if din > P:
                    wt = cpool.tile([P, (din // P) * dout], bf16, tag=f"w{l}")
                    for c in range(din // P):
                        nc.sync.dma_start(wt[:, c * dout:(c + 1) * dout],
                                          w_d[l][c * P:(c + 1) * P, :])
                else:
                    wt = cpool.tile([din, dout], bf16, tag=f"w{l}")
                    nc.sync.dma_start(wt[:], w_d[l][:])
                w_sb.append(wt)
                bt = cpool.tile([P, dout], fp32, tag=f"b{l}")
                nc.sync.dma_start(bt[:], b_d[l][:])
                b_sb.append(bt)

            zacc = wpool.tile([P, G * DIMS[0]], fp32, tag="zacc")
            ztab = wpool.tile([P, G * DIMS[0]], bf16, tag="ztab")
            zrelu = wpool.tile([P, G * DIMS[0]], bf16, tag="zrelu")
            # f32 fold scratch: w*ceil(K/8) <= 42 slots of up to 64 features
            fscr = wpool.tile([P, 42 * DIMS[0]], fp32, tag="fscr")

            tables = []
            shards = []
            for l in range(4):
                tables.append(dpool.tile([C * PC, DIMS[l]], bf16,
                                         name=f"table{l}", tag=f"table{l}",
                                         addr_space="Shared"))
                shards.append(dpool.tile([PC, DIMS[l]], bf16,
                                         name=f"shard{l}", tag=f"shard{l}"))

            def mm_group0(t):
                """layer-0 matmul for group t. x rows are pre-scaled by dinv
                on the host, so the psum->ztab copy is unscaled."""
                dout = dims[1]
                ps = pspool.tile([P, dout], fp32, tag="mmps")
                for cch in range(IN_DIM // P):
                    nc.tensor.matmul(
                        ps[:],
                        lhsT=xfull[cch][:, t * P:(t + 1) * P],
                        rhs=w_sb[0][:, cch * dout:(cch + 1) * dout],
                        start=(cch == 0), stop=(cch == IN_DIM // P - 1))
                nc.scalar.activation(ztab[:, t * dout:(t + 1) * dout], ps[:],
                                     AF.Copy)

            def mm_chain(l, t, eff):
                """layers 1-3: one transpose+matmul chain covering groups
                [t, t+eff) via a block-diagonal weight. zrelu holds
                zs = dinv*z so no per-group scale is needed."""
                din, dout = dims[l], dims[l + 1]
                pst = pstpool.tile([eff * din, P], bf16, tag="trps")
                nc.tensor.transpose(
                    pst[:], zrelu[:, t * din:(t + eff) * din], ident[:])
                zT = mmpool.tile([eff * din, P], bf16, tag="zT")
                nc.scalar.activation(zT[:], pst[:], AF.Copy)
                ps = pspool.tile([P, eff * dout], fp32, tag="mmps")
                nc.tensor.matmul(
                    ps[:], lhsT=zT[:],
                    rhs=wblk_sb[l][:eff * din, :eff * dout],
                    start=True, stop=True)
                nc.scalar.activation(ztab[:, t * dout:(t + eff) * dout],
                                     ps[:], AF.Copy)

            def table_chunk(l, ci):
                """groups [g0, g1) of table l: matmuls + shard chunk write."""
                g0, g1 = CHUNKS[ci], CHUNKS[ci + 1]
                w = g1 - g0
                dout = dims[l + 1]
                if l == 0:
                    for t in range(g0, g1):
                        mm_group0(t)
                else:
                    t = g0
                    while t < g1:
                        eff = min(GB[l], g1 - t)
                        mm_chain(l, t, eff)
                        t += eff
                shard_ap = shards[l][P * g0:P * g1].rearrange(
                    "(p g) d -> p g d", g=w)
                nc.sync.dma_start(shard_ap,
                                  ztab[:, g0 * dout:g1 * dout].rearrange(
                                      "p (g d) -> p g d", d=dout))

            def table_ag(l):
                """single whole-table AllGather (Shared tensors allow only
                one writer instruction)."""
                nc.gpsimd.collective_compute(
                    "AllGather", ALU.bypass,
                    replica_groups=[list(range(C))],
                    ins=[shards[l].opt()], outs=[tables[l].opt()])

            def epilogue_range(l, g0, g1):
                """finalize zacc groups [g0, g1): z = relu(zacc*dinv + b),
                written to zrelu (bf16). With b==0 the scale+relu fuses on
                the Scalar engine per group; otherwise DVE ops."""
                dout = dims[l + 1]
                if b_zero:
                    # zs = dinv*relu(zacc*dinv) = relu(zacc*dinv^2)
                    for t in range(g0, g1):
                        nc.scalar.activation(
                            zrelu[:, t * dout:(t + 1) * dout],
                            zacc[:, t * dout:(t + 1) * dout],
                            AF.Relu, scale=dinv2_sb[:, t:t + 1])
                else:
                    za3 = zacc[:, g0 * dout:g1 * dout].rearrange(
                        "p (g d) -> p g d", d=dout)
                    dinv_bc = dinv_sb[:, g0:g1].unsqueeze(2).broadcast_to(
                        [P, g1 - g0, dout])
                    nc.vector.tensor_tensor(out=za3, in0=za3, in1=dinv_bc,
                                            op=ALU.mult)
                    b_bc = b_sb[l][:].unsqueeze(1).broadcast_to(
                        [P, g1 - g0, dout])
                    nc.vector.tensor_tensor(out=za3, in0=za3, in1=b_bc,
                                            op=ALU.add)
                    nc.vector.tensor_scalar_max(
                        zrelu[:, g0 * dout:g1 * dout],
                        zacc[:, g0 * dout:g1 * dout], 0.0)
                    zr3 = zrelu[:, g0 * dout:g1 * dout].rearrange(
                        "p (g d) -> p g d", d=dout)
                    nc.vector.tensor_tensor(out=zr3, in0=zr3, in1=dinv_bc,
                                            op=ALU.mult)

            def fold(l, si, gs, w, K, c0, msg, gth):
                """fold-tree segmented sum of one supergroup into zacc."""
                dout = dims[l + 1]
                m4 = msg[:, :w * K * dout].rearrange(
                    "p (w k d) -> p w k d", w=w, k=K)
                zv = zacc[:, gs * dout:(gs + w) * dout].rearrange(
                    "p (w k d) -> p w k d", w=w, k=1)
                if K == 1:
                    cp = nc.vector.tensor_copy(zv, m4[:, :, 0:1, :])
                    _add_dep_helper(cp.ins, gth.ins, sync=True,
                                    reason="fold waits gather data")
                    return
                cur = m4
                L = K
                lvl = 0
                first = True
                while L > 1:
                    h = (L + 1) // 2
                    n = L - h
                    if L == 2:
                        tt = nc.vector.tensor_tensor(
                            out=zv, in0=cur[:, :, 0:1, :],
                            in1=cur[:, :, 1:2, :], op=ALU.add)
                    elif lvl != 3:
                        # in place (leftover slot [n:h] stays put):
                        # bf16 for lvl 0-2, f32 scratch for lvl > 3
                        tt = nc.vector.tensor_tensor(
                            out=cur[:, :, 0:n, :], in0=cur[:, :, 0:n, :],
                            in1=cur[:, :, h:h + n, :], op=ALU.add)
                    else:
                        # lvl == 3: switch to the f32 scratch (k=h view,
                        # kept in place for all later levels)
                        f4 = fscr[:, :w * h * dout].rearrange(
                            "p (w k d) -> p w k d", w=w, k=h)
                        tt = nc.vector.tensor_tensor(
                            out=f4[:, :, 0:n, :], in0=cur[:, :, 0:n, :],
                            in1=cur[:, :, h:h + n, :], op=ALU.add)
                        if h > n:
                            nc.vector.tensor_copy(f4[:, :, n:h, :],
                                                  cur[:, :, n:h, :])
                        cur = f4
                    if first:
                        _add_dep_helper(tt.ins, gth.ins, sync=True,
                                        reason="fold waits gather data")
                        first = False
                    L = h
                    lvl += 1

            # idx col offset per supergroup
            col0 = [0]
            for (gs, w, K) in sgs:
                col0.append(col0[-1] + w * K)

            red = wpool.tile([P, G], fp32, tag="red")
            exps = wpool.tile([P, G * 4], fp32, tag="exps")
            logs = wpool.tile([P, G], fp32, tag="logs")
            out_full = out_d.rearrange("(p g) d -> p g d", g=G)

            def tail_range(g0, g1):
                """final epilogue + log_softmax + output write for groups
                [g0, g1) (layer 3, d=4)."""
                wr = g1 - g0
                za3 = zacc[:, g0 * 4:g1 * 4].rearrange(
                    "p (g d) -> p g d", d=4)
                dinv_bc = dinv_sb[:, g0:g1].unsqueeze(2).broadcast_to(
                    [P, wr, 4])
                nc.vector.tensor_tensor(out=za3, in0=za3, in1=dinv_bc,
                                        op=ALU.mult)
                if not b_zero:
                    b_bc = b_sb[3][:].unsqueeze(1).broadcast_to([P, wr, 4])
                    nc.vector.tensor_tensor(out=za3, in0=za3, in1=b_bc,
                                            op=ALU.add)
                rd = red[:, g0:g1]
                nc.vector.tensor_reduce(out=rd, in_=za3,
                                        axis=mybir.AxisListType.X, op=ALU.max)
                rd_bc = rd.unsqueeze(2).broadcast_to([P, wr, 4])
                nc.vector.tensor_tensor(out=za3, in0=za3, in1=rd_bc,
                                        op=ALU.subtract)
                ex = exps[:, g0 * 4:g1 * 4]
                nc.scalar.activation(ex, zacc[:, g0 * 4:g1 * 4], AF.Exp)
                nc.vector.tensor_reduce(
                    out=rd, in_=ex.rearrange("p (g d) -> p g d", d=4),
                    axis=mybir.AxisListType.X, op=ALU.add)
                lg = logs[:, g0:g1]
                nc.scalar.activation(lg, rd, AF.Ln)
                lg_bc = lg.unsqueeze(2).broadcast_to([P, wr, 4])
                nc.vector.tensor_tensor(out=za3, in0=za3, in1=lg_bc,
                                        op=ALU.subtract)
                nc.sync.dma_start(out_full[:, g0:g1, :], za3)

            # ---------------- pipeline ----------------
            # layer-0 table: two bulk xT loads + 4 chunks (matmul + AG)
            xfull = []
            for cch in range(IN_DIM // P):
                xt = mpool.tile([P, PC], bf16, tag="msg", name=f"xf{cch}")
                nc.sync.dma_start(xt[:], xT[cch * P:(cch + 1) * P, :])
                xfull.append(xt)
            for ci in range(NCH):
                table_chunk(0, ci)
            table_ag(0)

            # merge consecutive supergroups into one gather while the slots
            # fit the msg tile (DIMS[0]*S0 elements)
            MF = {0: 1, 1: 2, 2: 4, 3: 8}
            for l in range(4):
                dout = dims[l + 1]
                emitted = 0   # chunks of table l+1 already emitted
                runs = []     # [a, b): sg index ranges per gather
                # keep the last few supergroups solo so their folds (and the
                # table-chunk emissions they gate) pipeline at fine grain
                limit = max(0, len(sgs) - 3)
                a = 0
                while a < len(sgs):
                    b = a + 1
                    while (b < limit and b - a < MF[l] and
                           (col0[b + 1] - col0[a]) * dout
                           <= MSG_SLOT_BUDGET * DIMS[0]):
                        b += 1
                    runs.append((a, b))
                    a = b
                for (a, b) in runs:
                    cols = col0[b] - col0[a]
                    msg = mpool.tile([P, MSG_SLOT_BUDGET * DIMS[0]], bf16,
                                     tag="msg")
                    gth = nc.gpsimd.indirect_dma_start(
                        out=msg[:, :cols * dout],
                        out_offset=None,
                        in_=tables[l][:],
                        in_offset=bass.IndirectOffsetOnAxis(
                            ap=idx_sb[:, col0[a]:col0[b]], axis=0),
                    )
                    for si in range(a, b):
                        gs, w, K = sgs[si]
                        base = (col0[si] - col0[a]) * dout
                        fold(l, si, gs, w, K, col0[si],
                             msg[:, base:base + w * K * dout], gth)
                        covered = gs + w
                        while (emitted < NCH and
                               CHUNKS[emitted + 1] <= covered):
                            lo, hi = CHUNKS[emitted], CHUNKS[emitted + 1]
                            if l < 3:
                                epilogue_range(l, lo, hi)
                                table_chunk(l + 1, emitted)
                            else:
                                tail_range(lo, hi)
                            emitted += 1
                            if emitted == NCH and l < 3:
                                table_ag(l + 1)

    nc.compile()
    return nc


# ----------------------------------------------------------------------------
# Entry point
# ----------------------------------------------------------------------------
def kernel(x, edge_index, W1, b1, W2, b2, W3, b3, W4, b4):
    global LAST_RESULT
    from concourse.bass_utils import run_bass_kernel_spmd

    prep = _preprocess(np.asarray(edge_index))
    sgs = tuple(prep["sgs"])
    SK = prep["SK"]

    Ws = [np.asarray(w, dtype=np.float32) for w in (W1, W2, W3, W4)]
    bs = [np.asarray(b, dtype=np.float32) for b in (b1, b2, b3, b4)]
    b_zero = not any(np.any(b) for b in bs)

    key = (sgs, SK, b_zero)
    if key not in _COMPILED:
        _COMPILED[key] = _build_program(list(sgs), SK, b_zero)
    nc = _COMPILED[key]

    x = np.asarray(x, dtype=np.float32)
    old_of_row = prep["old_of_row"]

    def blkdiag(Wm, nb):
        din, dout = Wm.shape
        B = np.zeros((nb * din, nb * dout), dtype=np.float32)
        for i in range(nb):
            B[i * din:(i + 1) * din, i * dout:(i + 1) * dout] = Wm
        return B

    # xT columns are pos-ordered (pos = g*P + p), matching matmul group
    # tiles; rows pre-scaled by dinv so table0 = (dinv*x) @ W1
    new_id = prep["new_id"]
    dinv_old = prep["dinv"]
    in_maps = []
    for k in range(C):
        xk = np.zeros((PC, IN_DIM), dtype=np.float32)
        mask = (new_id // PC) == k
        xk[new_id[mask] % PC] = x[mask] * dinv_old[mask][:, None]
        im = {
            "xT": np.ascontiguousarray(xk.T).astype(ml_dtypes.bfloat16),
            "idx": prep["IDX"][k],
            "dinv": prep["dinv_arr"][k],
            "dinv2": prep["dinv_arr"][k] ** 2,
            "wblk2": blkdiag(Ws[1], 2).astype(ml_dtypes.bfloat16),
            "wblk3": blkdiag(Ws[2], 4).astype(ml_dtypes.bfloat16),
            "wblk4": blkdiag(Ws[3], 4).astype(ml_dtypes.bfloat16),
        }
        for l in range(4):
            im[f"w{l+1}"] = Ws[l].astype(ml_dtypes.bfloat16)
            im[f"b{l+1}"] = np.broadcast_to(bs[l][None, :],
                                            (P, bs[l].shape[0])).copy()
        in_maps.append(im)

    res = run_bass_kernel_spmd(nc, in_maps, core_ids=list(range(C)))
    LAST_RESULT = res

    out = np.zeros((N_NODES, 4), dtype=np.float32)
    for k in range(C):
        ok = np.asarray(res.results[k]["out"], dtype=np.float32)
        ids = old_of_row[k * PC:(k + 1) * PC]
        real = ids >= 0
        out[ids[real]] = ok[real]
    return out


# revision 49
# speedup vs baseline: 1.0172x; 1.0172x over previous
"""4-layer GCN (GroupGCN) on 8 TRN2 NeuronCores.

Strategy:
  - Nodes relabeled by degree rank and striped across the 8 cores (core = rank % 8)
    so every core owns 12500 nodes with an even mix of degrees; edges are assigned
    to the core that owns their dst node.
  - Per layer: each core computes the scaled feature table rows for its own nodes
    (h~ = dinv * (z @ W)) with PE matmuls; the table is distributed to all cores
    with CHUNKED AllGathers (4 group-range chunks, chunk-major DRAM rows) that
    are issued progressively from inside the previous layer's aggregation loop,
    so collectives and matmuls hide under the gather stream.
  - Aggregation: one indirect-DMA gather per supergroup (groups packed to a
    budget of 224 slots, no trailing padding) + a DVE fold-tree segmented sum
    (bf16 for 3 levels, f32 scratch after).
  - dinv (D^-1/2) factorizes out of the per-edge norm: table rows are pre-scaled
    by dinv[src], the aggregated sum is post-scaled by dinv[dst] (fused with the
    ReLU on the Scalar engine when biases are all-zero).
  - bf16 tables/matmuls, f32 accumulation, f32 log_softmax output.
"""

import numpy as np
import ml_dtypes

N_NODES = 100000
N_EDGES = 3200000
IN_DIM = 256
DIMS = [64, 32, 16, 4]
C = 8           # cores
P = 128         # partitions
G = 98          # node groups of 128 per core
PC = G * P      # padded nodes per core (12544)
NPC = N_NODES // C  # real nodes per core (12500)
MSG_SLOT_BUDGET = 224   # max slots (w*K) per supergroup gather
CHUNKS = [0, 24, 48, 70, 88, 98]   # table chunk boundaries (group ranges)
NCH = len(CHUNKS) - 1

_COMPILED = {}
LAST_RESULT = None


# ----------------------------------------------------------------------------
# Host-side graph preprocessing
# ----------------------------------------------------------------------------
def _row_of_pos():
    """DRAM table row for each (pos in [0, PC)): chunk-major layout.

    Chunk c spans groups [g0, g1); global rows for the chunk are
    [C*P*g0, C*P*g1) with row = C*P*g0 + core*P*w + p*w + (g-g0)."""
    pos = np.arange(PC)
    g = pos // P
    p = pos % P
    row = np.empty(PC, dtype=np.int64)
    for ci in range(len(CHUNKS) - 1):
        g0, g1 = CHUNKS[ci], CHUNKS[ci + 1]
        w = g1 - g0
        m = (g >= g0) & (g < g1)
        # core offset added later (core stride inside chunk is P*w)
        row[m] = C * P * g0 + p[m] * w + (g[m] - g0)
    return row  # per-core row; add core*P*w_chunk via chunk lookup


def _preprocess(edge_index):
    src = edge_index[0].astype(np.int64)
    dst = edge_index[1].astype(np.int64)
    loop = np.arange(N_NODES, dtype=np.int64)
    src = np.concatenate([src, loop])
    dst = np.concatenate([dst, loop])

    deg = np.bincount(dst, minlength=N_NODES).astype(np.float64)
    dinv = np.where(deg > 0, 1.0 / np.sqrt(deg), 0.0).astype(np.float32)

    order = np.argsort(deg, kind="stable")          # ascending degree
    core_of_rank = np.arange(N_NODES) % C
    pos_of_rank = np.arange(N_NODES) // C
    new_id = np.empty(N_NODES, dtype=np.int64)      # (c, pos) packed
    new_id[order] = core_of_rank * PC + pos_of_rank

    # global table row for every (core, pos)
    pos_all = np.arange(C * PC) % PC
    core_all = np.arange(C * PC) // PC
    g_all = pos_all // P
    p_all = pos_all % P
    row_all = np.empty(C * PC, dtype=np.int64)
    for ci in range(len(CHUNKS) - 1):
        g0, g1 = CHUNKS[ci], CHUNKS[ci + 1]
        w = g1 - g0
        m = (g_all >= g0) & (g_all < g1)
        row_all[m] = (C * P * g0 + core_all[m] * P * w + p_all[m] * w
                      + (g_all[m] - g0))

    old_of_row = np.full(C * PC, -1, dtype=np.int64)
    old_of_row[row_all[new_id]] = np.arange(N_NODES)

    src_n = new_id[src]
    dst_n = new_id[dst]

    sort_idx = np.lexsort((src_n, dst_n))
    src_n = src_n[sort_idx]
    dst_n = dst_n[sort_idx]

    deg_n = np.bincount(dst_n, minlength=C * PC)
    deg_pg = deg_n.reshape(C, G, P)
    K_g = np.maximum(deg_pg.max(axis=(0, 2)), 1).astype(np.int64)  # [G]

    # pack groups into supergroups of at most S0 = w*K slots (compact: the
    # gather width is exactly w*K, no budget padding)
    S0 = MSG_SLOT_BUDGET
    sgs = []  # list of (g_start, width, K)
    g0 = 0
    while g0 < G:
        K = int(K_g[g0])
        w = 1
        while (g0 + w < G and
               max(K, int(K_g[g0 + w])) * (w + 1) <= S0 and
               w < 14):
            K = max(K, int(K_g[g0 + w]))
            w += 1
        K = max(K, int(K_g[g0:g0 + w].max()))
        sgs.append((g0, w, K))
        g0 += w

    col0 = np.zeros(len(sgs) + 1, dtype=np.int64)   # idx col offset per sg
    for si, (gs, w, K) in enumerate(sgs):
        col0[si + 1] = col0[si] + w * K
    SK = int(col0[-1])
    col_of_group = np.zeros(G, dtype=np.int64)
    for si, (gs, w, K) in enumerate(sgs):
        for i in range(w):
            col_of_group[gs + i] = col0[si] + i * K

    # zero row: first dummy node of core 0 (table rows of dummies are zeroed
    # by their dinv=0 pre-scale)
    ZERO_ROW = int(row_all[NPC])

    IDX = np.full((C, P, SK), ZERO_ROW, dtype=np.int32)
    node_starts = np.zeros(C * PC + 1, dtype=np.int64)
    np.cumsum(deg_n, out=node_starts[1:])
    slot_in_node = np.arange(len(dst_n)) - node_starts[dst_n]
    core_e = dst_n // PC
    pos_e = dst_n % PC
    p_e = pos_e % P
    g_e = pos_e // P
    col_e = col_of_group[g_e] + slot_in_node
    IDX[core_e, p_e, col_e] = row_all[src_n].astype(np.int32)

    dinv_n = np.zeros(C * PC, dtype=np.float32)
    dinv_n[new_id] = dinv
    dinv_arr = dinv_n.reshape(C, G, P).transpose(0, 2, 1).copy()  # [C, P, G]

    return dict(
        dinv_arr=dinv_arr, IDX=IDX, sgs=sgs, SK=SK,
        old_of_row=old_of_row, new_id=new_id, dinv=dinv,
    )


# ----------------------------------------------------------------------------
# Device kernel builder
# ----------------------------------------------------------------------------
def _build_program(sgs, SK, b_zero):
    import concourse.bass as bass
    import concourse.mybir as mybir
    import concourse.tile as tile
    import concourse.bacc as bacc
    from concourse.masks import make_identity
    from concourse.bass import _add_dep_helper

    fp32 = mybir.dt.float32
    bf16 = mybir.dt.bfloat16
    i32 = mybir.dt.int32
    AF = mybir.ActivationFunctionType
    ALU = mybir.AluOpType

    nc = bacc.Bacc("TRN2", target_bir_lowering=False, debug=False,
                   enable_asserts=False, num_devices=C)

    dims = [IN_DIM] + DIMS
    GB = {1: 2, 2: 4, 3: 4}   # matmul group-batch per layer (block-diag W)
    xT = nc.dram_tensor("xT", [IN_DIM, PC], bf16, kind="ExternalInput").ap()
    idx_d = nc.dram_tensor("idx", [P, SK], i32, kind="ExternalInput").ap()
    dinv_d = nc.dram_tensor("dinv", [P, G], fp32, kind="ExternalInput").ap()
    dinv2_d = nc.dram_tensor("dinv2", [P, G], fp32, kind="ExternalInput").ap()
    wblk_d = {}
    for l in (1, 2, 3):
        wblk_d[l] = nc.dram_tensor(
            f"wblk{l+1}", [GB[l] * dims[l], GB[l] * dims[l + 1]], bf16,
            kind="ExternalInput").ap()
    w_d = []
    b_d = []
    for l in range(4):
        w_d.append(nc.dram_tensor(f"w{l+1}", [dims[l], dims[l + 1]], bf16,
                                  kind="ExternalInput").ap())
        b_d.append(nc.dram_tensor(f"b{l+1}", [P, dims[l + 1]], fp32,
                                  kind="ExternalInput").ap())
    out_d = nc.dram_tensor("out", [PC, 4], fp32, kind="ExternalOutput").ap()

    with tile.TileContext(nc) as tc:
        with (
            tc.tile_pool(name="const", bufs=1) as cpool,
            tc.tile_pool(name="work", bufs=1) as wpool,
            tc.tile_pool(name="msg", bufs=3) as mpool,
            tc.tile_pool(name="mm", bufs=3) as mmpool,
            tc.tile_pool(name="psum", bufs=4, space="PSUM") as pspool,
            tc.tile_pool(name="pst", bufs=4, space="PSUM") as pstpool,
            tc.tile_pool(name="dram", bufs=1, space="DRAM") as dpool,
        ):
            # ---- constants ----
            dinv_sb = cpool.tile([P, G], fp32)
            nc.sync.dma_start(dinv_sb[:], dinv_d[:])
            dinv2_sb = cpool.tile([P, G], fp32)
            nc.sync.dma_start(dinv2_sb[:], dinv2_d[:])
            wblk_sb = {}
            for l in (1, 2, 3):
                wblk_sb[l] = cpool.tile(
                    [GB[l] * dims[l], GB[l] * dims[l + 1]], bf16,
                    tag=f"wblk{l}", name=f"wblk{l}")
                nc.sync.dma_start(wblk_sb[l][:], wblk_d[l][:])
            idx_sb = cpool.tile([P, SK], i32, tag="idxsb")
            nc.sync.dma_start(idx_sb[:], idx_d[:])
            ident = cpool.tile([P, P], bf16)
            make_identity(nc, ident[:])
            w_sb = []
            b_sb = []
            for l in range(4):
                din, dout = dims[l], dims[l + 1]
                if din > P:
                    wt = cpool.tile([P, (din // P) * dout], bf16, tag=f"w{l}")
                    for c in range(din // P):
                        nc.sync.dma_start(wt[:, c * dout:(c + 1) * dout],
                                          w_d[l][c * P:(c + 1) * P, :])
                else:
                    wt = cpool.tile([din, dout], bf16, tag=f"w{l}")
                    nc.sync.dma_start(wt[:], w_d[l][:])
                w_sb.append(wt)
                bt = cpool.tile([P, dout], fp32, tag=f"b{l}")
                nc.sync.dma_start(bt[:], b_d[l][:])
                b_sb.append(bt)

            zacc = wpool.tile([P, G * DIMS[0]], fp32, tag="zacc")
            ztab = wpool.tile([P, G * DIMS[0]], bf16, tag="ztab")
            zrelu = wpool.tile([P, G * DIMS[0]], bf16, tag="zrelu")
            # f32 fold scratch: w*ceil(K/8) <= 42 slots of up to 64 features
            fscr = wpool.tile([P, 42 * DIMS[0]], fp32, tag="fscr")

            tables = []
            shards = []
            for l in range(4):
                tables.append(dpool.tile([C * PC, DIMS[l]], bf16,
                                         name=f"table{l}", tag=f"table{l}",
                                         addr_space="Shared"))
                shards.append(dpool.tile([PC, DIMS[l]], bf16,
                                         name=f"shard{l}", tag=f"shard{l}"))

            def mm_group0(t):
                """layer-0 matmul for group t. x rows are pre-scaled by dinv
                on the host, so the psum->ztab copy is unscaled."""
                dout = dims[1]
                ps = pspool.tile([P, dout], fp32, tag="mmps")
                for cch in range(IN_DIM // P):
                    nc.tensor.matmul(
                        ps[:],
                        lhsT=xfull[cch][:, t * P:(t + 1) * P],
                        rhs=w_sb[0][:, cch * dout:(cch + 1) * dout],
                        start=(cch == 0), stop=(cch == IN_DIM // P - 1))
                nc.scalar.activation(ztab[:, t * dout:(t + 1) * dout], ps[:],
                                     AF.Copy)

            def mm_chain(l, t, eff):
                """layers 1-3: one transpose+matmul chain covering groups
                [t, t+eff) via a block-diagonal weight. zrelu holds
                zs = dinv*z so no per-group scale is needed."""
                din, dout = dims[l], dims[l + 1]
                pst = pstpool.tile([eff * din, P], bf16, tag="trps")
                nc.tensor.transpose(
                    pst[:], zrelu[:, t * din:(t + eff) * din], ident[:])
                zT = mmpool.tile([eff * din, P], bf16, tag="zT")
                nc.scalar.activation(zT[:], pst[:], AF.Copy)
                ps = pspool.tile([P, eff * dout], fp32, tag="mmps")
                nc.tensor.matmul(
                    ps[:], lhsT=zT[:],
                    rhs=wblk_sb[l][:eff * din, :eff * dout],
                    start=True, stop=True)
                nc.scalar.activation(ztab[:, t * dout:(t + eff) * dout],
                                     ps[:], AF.Copy)

            def table_chunk(l, ci):
                """groups [g0, g1) of table l: matmuls + shard chunk write."""
                g0, g1 = CHUNKS[ci], CHUNKS[ci + 1]
                w = g1 - g0
                dout = dims[l + 1]
                if l == 0:
                    for t in range(g0, g1):
                        mm_group0(t)
                else:
                    t = g0
                    while t < g1:
                        eff = min(GB[l], g1 - t)
                        mm_chain(l, t, eff)
                        t += eff
                shard_ap = shards[l][P * g0:P * g1].rearrange(
                    "(p g) d -> p g d", g=w)
                nc.sync.dma_start(shard_ap,
                                  ztab[:, g0 * dout:g1 * dout].rearrange(
                                      "p (g d) -> p g d", d=dout))

            def table_ag(l):
                """single whole-table AllGather (Shared tensors allow only
                one writer instruction)."""
                nc.gpsimd.collective_compute(
                    "AllGather", ALU.bypass,
                    replica_groups=[list(range(C))],
                    ins=[shards[l].opt()], outs=[tables[l].opt()])

            def epilogue_range(l, g0, g1):
                """finalize zacc groups [g0, g1): z = relu(zacc*dinv + b),
                written to zrelu (bf16). With b==0 the scale+relu fuses on
                the Scalar engine per group; otherwise DVE ops."""
                dout = dims[l + 1]
                if b_zero:
                    # zs = dinv*relu(zacc*dinv) = relu(zacc*dinv^2)
                    for t in range(g0, g1):
                        nc.scalar.activation(
                            zrelu[:, t * dout:(t + 1) * dout],
                            zacc[:, t * dout:(t + 1) * dout],
                            AF.Relu, scale=dinv2_sb[:, t:t + 1])
                else:
                    za3 = zacc[:, g0 * dout:g1 * dout].rearrange(
                        "p (g d) -> p g d", d=dout)
                    dinv_bc = dinv_sb[:, g0:g1].unsqueeze(2).broadcast_to(
                        [P, g1 - g0, dout])
                    nc.vector.tensor_tensor(out=za3, in0=za3, in1=dinv_bc,
                                            op=ALU.mult)
                    b_bc = b_sb[l][:].unsqueeze(1).broadcast_to(
                        [P, g1 - g0, dout])
                    nc.vector.tensor_tensor(out=za3, in0=za3, in1=b_bc,
                                            op=ALU.add)
                    nc.vector.tensor_scalar_max(
                        zrelu[:, g0 * dout:g1 * dout],
                        zacc[:, g0 * dout:g1 * dout], 0.0)
                    zr3 = zrelu[:, g0 * dout:g1 * dout].rearrange(
                        "p (g d) -> p g d", d=dout)
                    nc.vector.tensor_tensor(out=zr3, in0=zr3, in1=dinv_bc,
                                            op=ALU.mult)

            def fold(l, si, gs, w, K, c0, msg, gth):
                """fold-tree segmented sum of one supergroup into zacc."""
                dout = dims[l + 1]
                m4 = msg[:, :w * K * dout].rearrange(
                    "p (w k d) -> p w k d", w=w, k=K)
                zv = zacc[:, gs * dout:(gs + w) * dout].rearrange(
                    "p (w k d) -> p w k d", w=w, k=1)
                if K == 1:
                    cp = nc.vector.tensor_copy(zv, m4[:, :, 0:1, :])
                    _add_dep_helper(cp.ins, gth.ins, sync=True,
                                    reason="fold waits gather data")
                    return
                cur = m4
                L = K
                lvl = 0
                first = True
                while L > 1:
                    h = (L + 1) // 2
                    n = L - h
                    if L == 2:
                        tt = nc.vector.tensor_tensor(
                            out=zv, in0=cur[:, :, 0:1, :],
                            in1=cur[:, :, 1:2, :], op=ALU.add)
                    elif lvl != 3:
                        # in place (leftover slot [n:h] stays put):
                        # bf16 for lvl 0-2, f32 scratch for lvl > 3
                        tt = nc.vector.tensor_tensor(
                            out=cur[:, :, 0:n, :], in0=cur[:, :, 0:n, :],
                            in1=cur[:, :, h:h + n, :], op=ALU.add)
                    else:
                        # lvl == 3: switch to the f32 scratch (k=h view,
                        # kept in place for all later levels)
                        f4 = fscr[:, :w * h * dout].rearrange(
                            "p (w k d) -> p w k d", w=w, k=h)
                        tt = nc.vector.tensor_tensor(
                            out=f4[:, :, 0:n, :], in0=cur[:, :, 0:n, :],
                            in1=cur[:, :, h:h + n, :], op=ALU.add)
                        if h > n:
                            nc.vector.tensor_copy(f4[:, :, n:h, :],
                                                  cur[:, :, n:h, :])
                        cur = f4
                    if first:
                        _add_dep_helper(tt.ins, gth.ins, sync=True,
                                        reason="fold waits gather data")
                        first = False
                    L = h
                    lvl += 1

            # idx col offset per supergroup
            col0 = [0]
            for (gs, w, K) in sgs:
                col0.append(col0[-1] + w * K)

            red = wpool.tile([P, G], fp32, tag="red")
            exps = wpool.tile([P, G * 4], fp32, tag="exps")
            logs = wpool.tile([P, G], fp32, tag="logs")
            out_full = out_d.rearrange("(p g) d -> p g d", g=G)

            def tail_range(g0, g1):
                """final epilogue + log_softmax + output write for groups
                [g0, g1) (layer 3, d=4)."""
                wr = g1 - g0
                za3 = zacc[:, g0 * 4:g1 * 4].rearrange(
                    "p (g d) -> p g d", d=4)
                dinv_bc = dinv_sb[:, g0:g1].unsqueeze(2).broadcast_to(
                    [P, wr, 4])
                nc.vector.tensor_tensor(out=za3, in0=za3, in1=dinv_bc,
                                        op=ALU.mult)
                if not b_zero:
                    b_bc = b_sb[3][:].unsqueeze(1).broadcast_to([P, wr, 4])
                    nc.vector.tensor_tensor(out=za3, in0=za3, in1=b_bc,
                                            op=ALU.add)
                rd = red[:, g0:g1]
                nc.vector.tensor_reduce(out=rd, in_=za3,
                                        axis=mybir.AxisListType.X, op=ALU.max)
                rd_bc = rd.unsqueeze(2).broadcast_to([P, wr, 4])
                nc.vector.tensor_tensor(out=za3, in0=za3, in1=rd_bc,
                                        op=ALU.subtract)
                ex = exps[:, g0 * 4:g1 * 4]
                nc.scalar.activation(ex, zacc[:, g0 * 4:g1 * 4], AF.Exp)
                nc.vector.tensor_reduce(
                    out=rd, in_=ex.rearrange("p (g d) -> p g d", d=4),
                    axis=mybir.AxisListType.X, op=ALU.add)
                lg = logs[:, g0:g1]
                nc.scalar.activation(lg, rd, AF.Ln)
                lg_bc = lg.unsqueeze(2).broadcast_to([P, wr, 4])
                nc.vector.tensor_tensor(out=za3, in0=za3, in1=lg_bc,
                                        op=ALU.subtract)
                nc.sync.dma_start(out_full[:, g0:g1, :], za3)

            # ---------------- pipeline ----------------
            # layer-0 table: two bulk xT loads + 4 chunks (matmul + AG)
            xfull = []
            for cch in range(IN_DIM // P):
                xt = mpool.tile([P, PC], bf16, tag="msg", name=f"xf{cch}")
                nc.sync.dma_start(xt[:], xT[cch * P:(cch + 1) * P, :])
                xfull.append(xt)
            for ci in range(NCH):
                table_chunk(0, ci)
            table_ag(0)

            for l in range(4):
                dout = dims[l + 1]
                emitted = 0   # chunks of table l+1 already emitted
                covered = 0
                for si, (gs, w, K) in enumerate(sgs):
                    msg = mpool.tile([P, MSG_SLOT_BUDGET * dout], bf16,
                                     tag="msg")
                    gth = nc.gpsimd.indirect_dma_start(
                        out=msg[:, :w * K * dout],
                        out_offset=None,
                        in_=tables[l][:],
                        in_offset=bass.IndirectOffsetOnAxis(
                            ap=idx_sb[:, col0[si]:col0[si + 1]], axis=0),
                    )
                    fold(l, si, gs, w, K, col0[si], msg, gth)
                    covered = gs + w
                    while (emitted < NCH and
                           CHUNKS[emitted + 1] <= covered):
                        lo, hi = CHUNKS[emitted], CHUNKS[emitted + 1]
                        if l < 3:
                            epilogue_range(l, lo, hi)
                            table_chunk(l + 1, emitted)
                        else:
                            tail_range(lo, hi)
                        emitted += 1
                        if emitted == NCH and l < 3:
                            table_ag(l + 1)
                while emitted < NCH:
                    lo, hi = CHUNKS[emitted], CHUNKS[emitted + 1]
                    if l < 3:
                        epilogue_range(l, lo, hi)
                        table_chunk(l + 1, emitted)
                    else:
                        tail_range(lo, hi)
                    emitted += 1
                    if emitted == NCH and l < 3:
                        table_ag(l + 1)

    nc.compile()
    return nc


# ----------------------------------------------------------------------------
# Entry point
# ----------------------------------------------------------------------------
def kernel(x, edge_index, W1, b1, W2, b2, W3, b3, W4, b4):
    global LAST_RESULT
    from concourse.bass_utils import run_bass_kernel_spmd

    prep = _preprocess(np.asarray(edge_index))
    sgs = tuple(prep["sgs"])
    SK = prep["SK"]

    Ws = [np.asarray(w, dtype=np.float32) for w in (W1, W2, W3, W4)]
    bs = [np.asarray(b, dtype=np.float32) for b in (b1, b2, b3, b4)]
    b_zero = not any(np.any(b) for b in bs)

    key = (sgs, SK, b_zero)
    if key not in _COMPILED:
        _COMPILED[key] = _build_program(list(sgs), SK, b_zero)
    nc = _COMPILED[key]

    x = np.asarray(x, dtype=np.float32)
    old_of_row = prep["old_of_row"]

    def blkdiag(Wm, nb):
        din, dout = Wm.shape
        B = np.zeros((nb * din, nb * dout), dtype=np.float32)
        for i in range(nb):
            B[i * din:(i + 1) * din, i * dout:(i + 1) * dout] = Wm
        return B

    # xT columns are pos-ordered (pos = g*P + p), matching matmul group
    # tiles; rows pre-scaled by dinv so table0 = (dinv*x) @ W1
    new_id = prep["new_id"]
    dinv_old = prep["dinv"]
    in_maps = []
    for k in range(C):
        xk = np.zeros((PC, IN_DIM), dtype=np.float32)
        mask = (new_id // PC) == k
        xk[new_id[mask] % PC] = x[mask] * dinv_old[mask][:, None]
        im = {
            "xT": np.ascontiguousarray(xk.T).astype(ml_dtypes.bfloat16),
            "idx": prep["IDX"][k],
            "dinv": prep["dinv_arr"][k],
            "dinv2": prep["dinv_arr"][k] ** 2,
            "wblk2": blkdiag(Ws[1], 2).astype(ml_dtypes.bfloat16),
            "wblk3": blkdiag(Ws[2], 4).astype(ml_dtypes.bfloat16),
            "wblk4": blkdiag(Ws[3], 4).astype(ml_dtypes.bfloat16),
        }
        for l in range(4):
            im[f"w{l+1}"] = Ws[l].astype(ml_dtypes.bfloat16)
            im[f"b{l+1}"] = np.broadcast_to(bs[l][None, :],
                                            (P, bs[l].shape[0])).copy()
        in_maps.append(im)

    res = run_bass_kernel_spmd(nc, in_maps, core_ids=list(range(C)))
    LAST_RESULT = res

    out = np.zeros((N_NODES, 4), dtype=np.float32)
    for k in range(C):
        ok = np.asarray(res.results[k]["out"], dtype=np.float32)
        ids = old_of_row[k * PC:(k + 1) * PC]
        real = ids >= 0
        out[ids[real]] = ok[real]
    return out
